# revision 40
# baseline (speedup 1.0000x reference)
"""Trainium2 Bass kernel for a transformer decoder block.

Shapes (hardcoded): B=4, S=1024, D=1024, H=16 heads, DH=64, FFN F=4096.

Sharding: 8 cores = 4 batches x 2 sequence-halves.  Core (b, h) handles
query rows {64*(2t+h)+r : t in 0..7, r in 0..63} of batch b (interleaved
64-row blocks so the causal-attention work per core is identical -> one
uniform SPMD program).  Each core recomputes the (small) K/V projections
it needs, so no collectives are required.

On-chip layout is feature-major ("transposed"): activations live as
[feature, token] so every matmul contraction sits on the partition axis.
The host pre-transposes inputs/weights and re-transposes the output.

Scheduling: engines execute their instruction streams in order, so each
attention head-pair's softmax (ScalarE-bound) is emitted with "filler"
projection matmul groups for the next head pair woven between its
k-chunks, keeping the PE busy while exps drain.
"""

import sys

if "/opt/trn_rl_repo" not in sys.path:
    sys.path.insert(0, "/opt/trn_rl_repo")

import numpy as np
import ml_dtypes

B, S, D, H, F, DH = 4, 1024, 1024, 16, 4096, 64
NCORES = 8
SQ = 512            # query rows per core
NDT = D // 128      # 8 d-tiles
NFT = F // 128      # 32 f-tiles
NHP = H // 2        # 8 head pairs
NKC = S // 128      # 8 k chunks
BF16 = ml_dtypes.bfloat16

_PROG = None


def _build_program():
    import concourse.mybir as mybir
    from concourse import bacc
    from concourse.tile import TileContext

    f32 = mybir.dt.float32
    bf16 = mybir.dt.bfloat16
    f32r = mybir.dt.float32r
    AF = mybir.ActivationFunctionType
    OP = mybir.AluOpType

    nc = bacc.Bacc("TRN2", target_bir_lowering=False, debug=False,
                   num_devices=NCORES)

    def din(name, shape, dt=bf16):
        return nc.dram_tensor(name, shape, dt, kind="ExternalInput")

    # activations, partition-major so each loads with ONE contiguous DMA
    xt_full = din("xt_full", [128, NDT, S])          # X^T (K/V source)
    xq = din("xq", [128, NDT, SQ])                   # X^T own q rows
    xr = din("xr", [128, NDT, SQ], f32)              # residual (f32)
    enc_t = din("enc_t", [128, NDT, S])              # encoder^T
    sa_mask = din("sa_mask", [128, NKC, 64])         # causal boundary slabs

    # weights staged host-side in exactly the sbuf tile layout
    w_sa_q = din("w_sa_q", [NHP, 128, NDT, 128])
    w_sa_k = din("w_sa_k", [NHP, 128, NDT, 128])
    w_sa_v = din("w_sa_v", [2, 128, NDT, 512])
    w_sa_o = din("w_sa_o", [NDT, 128, NDT, 128])
    w_ca_q = din("w_ca_q", [NHP, 128, NDT, 128])
    w_ca_k = din("w_ca_k", [NHP, 128, NDT, 128])
    w_ca_v = din("w_ca_v", [2, 128, NDT, 512])
    w_ca_o = din("w_ca_o", [NDT, 128, NDT, 128])
    w_ff1 = din("w_ff1", [NFT, 128, NDT, 128])
    w_ff2 = din("w_ff2", [NDT, 128, NFT, 128])

    # all small per-feature vectors concatenated: one DMA
    # cols: bq1 0:8 | bq2 8:16 | bo1 16:24 | bo2 24:32 | b2 32:40 |
    #       ln1g 40:48 | ln1b 48:56 | ln2g .. | ln3b 72:88 | b1 88:120
    NV = 120
    v_all = din("v_all", [128, NV], f32)

    out_t = nc.dram_tensor("out_t", [NDT, 128, SQ], f32, kind="ExternalOutput")

    with TileContext(nc) as tc:
        with tc.tile_pool(name="p_acc", bufs=2, space="PSUM") as p_acc, \
             tc.tile_pool(name="p_s", bufs=1, space="PSUM") as p_s, \
             tc.tile_pool(name="p_pav", bufs=2, space="PSUM") as p_pav, \
             tc.tile_pool(name="p_one", bufs=1, space="PSUM") as p_one, \
             tc.tile_pool(name="const", bufs=1) as cpool, \
             tc.tile_pool(name="big", bufs=1) as big, \
             tc.tile_pool(name="wcol", bufs=4) as wcol, \
             tc.tile_pool(name="wbig", bufs=2) as wbig, \
             tc.tile_pool(name="pt", bufs=4) as ptp, \
             tc.tile_pool(name="bc", bufs=2) as bcp, \
             tc.tile_pool(name="sm", bufs=1) as smp, \
             tc.tile_pool(name="tmp", bufs=2) as tmpp, \
             tc.tile_pool(name="outp", bufs=2) as outp:

            # ---------------- constants / small vectors ----------------
            ones_f = cpool.tile([128, 1], f32)
            nc.vector.memset(ones_f[:], 1.0)
            ones16 = cpool.tile([128, 1], bf16)
            nc.vector.tensor_copy(ones16[:], ones_f[:])
            # LN stat matmuls use 1/D so psum rows are mean / E[x^2] directly
            oned_f = cpool.tile([128, 1], f32)
            nc.vector.memset(oned_f[:], 1.0 / D)
            ones_r = cpool.tile([128, 1], f32r)
            nc.vector.tensor_copy(ones_r[:], oned_f[:])
            eps_t = cpool.tile([1, 1], f32)
            nc.vector.memset(eps_t[:], 1e-12)

            VA = cpool.tile([128, NV], f32)
            nc.sync.dma_start(out=VA[:], in_=v_all[:])
            bq1_sb, bq2_sb = VA[:, 0:8], VA[:, 8:16]
            bo1_sb, bo2_sb = VA[:, 16:24], VA[:, 24:32]
            b2_sb = VA[:, 32:40]
            ln_sb = {j: (VA[:, 40 + 16 * (j - 1):48 + 16 * (j - 1)],
                         VA[:, 48 + 16 * (j - 1):56 + 16 * (j - 1)])
                     for j in (1, 2, 3)}
            b1_sb = VA[:, 88:120]

            MS = cpool.tile([128, NKC, 64], bf16)
            nc.sync.dma_start(out=MS[:], in_=sa_mask[:])

            # ---------------- activation loads (XQ first: QT needs it) ----
            XQ = big.tile([128, NDT, SQ], bf16, tag="outb")
            nc.sync.dma_start(out=XQ[:], in_=xq[:])
            XT = big.tile([128, NDT, S], bf16, tag="xt")

            # ---------------- filler-step builders ----------------
            # Each returned closure emits one psum matmul group; they are
            # woven between attention k-chunks to keep the PE fed while the
            # ScalarE runs the softmax exps.
            def q_steps(hp, wq_d, src_q, bq_sb, QT):
                def run():
                    wqt = wcol.tile([128, NDT, 128], bf16, tag="wcol")
                    nc.sync.dma_start(out=wqt[:], in_=wq_d[hp])
                    pq = p_acc.tile([128, SQ], f32, tag="acc")
                    for dt in range(NDT):
                        nc.tensor.matmul(pq[:], wqt[:, dt, :], src_q[:, dt, :],
                                         start=(dt == 0), stop=(dt == NDT - 1))
                    nc.vector.tensor_scalar_add(QT[:, hp, :], pq[:],
                                                bq_sb[:, hp:hp + 1])
                return [run]

            def k_steps(hp, wk_d, src_kv, KT):
                cell = {}

                def run_kh(kh):
                    def run():
                        if kh == 0:
                            cell["w"] = wcol.tile([128, NDT, 128], bf16,
                                                  tag="wcol", name="wkt")
                            nc.sync.dma_start(out=cell["w"][:], in_=wk_d[hp])
                        wkt = cell["w"]
                        pk = p_acc.tile([128, 512], f32, tag="acc")
                        for dt in range(NDT):
                            nc.tensor.matmul(
                                pk[:], wkt[:, dt, :],
                                src_kv[:, dt, 512 * kh:512 * (kh + 1)],
                                start=(dt == 0), stop=(dt == NDT - 1))
                        nc.vector.tensor_copy(
                            KT[:, hp, 512 * kh:512 * (kh + 1)], pk[:])
                    return run
                return [run_kh(0), run_kh(1)]

            def v_steps(g, wv_d, src_kv, V2):
                cell = {}

                def run_kc(kc):
                    def run():
                        if kc == 0:
                            cell["w"] = wbig.tile([128, NDT, 512], bf16,
                                                  tag="wbig", name="wvt")
                            nc.sync.dma_start(out=cell["w"][:], in_=wv_d[g])
                        wvt = cell["w"]
                        pv = p_acc.tile([128, 512], f32, tag="acc")
                        for dt in range(NDT):
                            nc.tensor.matmul(
                                pv[:], src_kv[:, dt, 128 * kc:128 * (kc + 1)],
                                wvt[:, dt, :],
                                start=(dt == 0), stop=(dt == NDT - 1))
                        nc.vector.tensor_copy(V2[:, kc, g, :], pv[:])
                    return run
                return [run_kc(kc) for kc in range(NKC)]

            def attention(hp, QT, KT, V2, ATTN, causal, fillers=()):
                pav = p_pav.tile([128, SQ], f32, tag="pav")
                pda = p_one.tile([128, SQ], f32, tag="pda")
                pdb = p_one.tile([128, SQ], f32, tag="pdb")
                g, m = hp // 4, hp % 4
                fillers = list(fillers)
                fi = 0
                for j in range(NKC):
                    n0 = 64 * j if causal else 0
                    s2 = p_s.tile([128, 2, SQ], f32, tag="s")
                    ks = slice(128 * j, 128 * (j + 1))
                    nc.tensor.matmul(s2[:, 0, n0:SQ], KT[0:64, hp, ks],
                                     QT[0:64, hp, n0:SQ], start=True, stop=True)
                    nc.tensor.matmul(s2[:, 1, n0:SQ], KT[64:128, hp, ks],
                                     QT[64:128, hp, n0:SQ], start=True,
                                     stop=True)
                    # one exp + one mask op for both heads: fewer cross-engine
                    # waits on the PE stream (waits break LDW pull-ahead)
                    pt2 = ptp.tile([128, 2, SQ], bf16, tag="pt")
                    nc.scalar.activation(out=pt2[:, :, n0:SQ],
                                         in_=s2[:, :, n0:SQ],
                                         func=AF.Exp, scale=0.125)
                    if causal:
                        nc.vector.tensor_mul(
                            pt2[:, :, n0:n0 + 64], pt2[:, :, n0:n0 + 64],
                            MS[:, j:j + 1, :].to_broadcast([128, 2, 64]))
                    # fillers go HERE (between scores and AV) so the PE chews
                    # on them while ScalarE exps this chunk
                    while fi < len(fillers) and fi * NKC < (j + 1) * len(fillers):
                        fillers[fi]()
                        fi += 1
                    st, sp = (j == 0), (j == NKC - 1)
                    nc.tensor.matmul(pav[0:64, n0:SQ],
                                     V2[:, j, g, 128 * m:128 * m + 64],
                                     pt2[:, 0, n0:SQ], start=st, stop=sp)
                    nc.tensor.matmul(pav[64:128, n0:SQ],
                                     V2[:, j, g, 128 * m + 64:128 * (m + 1)],
                                     pt2[:, 1, n0:SQ], start=st, stop=sp)
                    nc.tensor.matmul(pda[0:1, n0:SQ], ones16[:, 0:1],
                                     pt2[:, 0, n0:SQ], start=st, stop=sp)
                    nc.tensor.matmul(pdb[0:1, n0:SQ], ones16[:, 0:1],
                                     pt2[:, 1, n0:SQ], start=st, stop=sp)
                while fi < len(fillers):
                    fillers[fi]()
                    fi += 1
                ra = smp.tile([1, SQ], f32, tag="ra")
                rb = smp.tile([1, SQ], f32, tag="rb")
                nc.vector.reciprocal_approx_fast(out=ra[:], in_=pda[0:1, :])
                nc.vector.reciprocal_approx_fast(out=rb[:], in_=pdb[0:1, :])
                RA = bcp.tile([128, SQ], f32, tag="bc")
                RB = bcp.tile([128, SQ], f32, tag="bc")
                nc.gpsimd.partition_broadcast(RA[:], ra[:])
                nc.gpsimd.partition_broadcast(RB[:], rb[:])
                nc.vector.tensor_mul(ATTN[0:64, hp, :], pav[0:64, :],
                                     RA[0:64, :])
                nc.vector.tensor_mul(ATTN[64:128, hp, :], pav[64:128, :],
                                     RB[64:128, :])

            def ln_tail(pst1, pst2, y, ln_g, ln_b, out_bf, out_f32, dma_out):
                m1 = smp.tile([1, SQ], f32, tag="m1")
                nc.vector.tensor_copy(m1[:], pst1[0:1, :])  # mean (ones=1/D)
                MB = bcp.tile([128, SQ], f32, tag="bc")
                nc.gpsimd.partition_broadcast(MB[:], m1[:])
                sq1 = smp.tile([1, SQ], f32, tag="sq1")
                nc.vector.tensor_mul(sq1[:], m1[:], m1[:])
                varp = smp.tile([1, SQ], f32, tag="varp")
                nc.vector.tensor_sub(varp[:], pst2[0:1, :], sq1[:])
                sv = smp.tile([1, SQ], f32, tag="sv")
                nc.scalar.activation(out=sv[:], in_=varp[:], func=AF.Sqrt,
                                     bias=eps_t[:], scale=float(D) / (D - 1))
                rstd = smp.tile([1, SQ], f32, tag="rstd")
                nc.vector.reciprocal_approx_fast(out=rstd[:], in_=sv[:])
                RS = bcp.tile([128, SQ], f32, tag="bc")
                nc.gpsimd.partition_broadcast(RS[:], rstd[:])
                for dt in range(NDT):
                    t1 = tmpp.tile([128, SQ], f32, tag="lnt")
                    nc.vector.tensor_sub(t1[:], y[:, dt, :], MB[:])
                    nc.vector.tensor_mul(t1[:], t1[:], RS[:])
                    g_ap = ln_g[:, dt:dt + 1]
                    b_ap = ln_b[:, dt:dt + 1]
                    use_act = (dt % 2 == 1)  # split affine between ACT / DVE
                    if dma_out is not None:
                        od = outp.tile([128, SQ], f32, tag="od")
                        if use_act:
                            nc.scalar.activation(out=od[:], in_=t1[:],
                                                 func=AF.Identity,
                                                 bias=b_ap, scale=g_ap)
                        else:
                            nc.vector.tensor_scalar(od[:], t1[:], g_ap, b_ap,
                                                    OP.mult, OP.add)
                        nc.sync.dma_start(out=dma_out[dt], in_=od[:])
                    else:
                        if use_act:
                            nc.scalar.activation(out=out_f32[:, dt, :],
                                                 in_=t1[:], func=AF.Identity,
                                                 bias=b_ap, scale=g_ap)
                            nc.scalar.activation(out=out_bf[:, dt, :],
                                                 in_=t1[:], func=AF.Identity,
                                                 bias=b_ap, scale=g_ap)
                        else:
                            nc.vector.tensor_scalar(out_f32[:, dt, :], t1[:],
                                                    g_ap, b_ap,
                                                    OP.mult, OP.add)
                            nc.vector.tensor_copy(out_bf[:, dt, :],
                                                  out_f32[:, dt, :])

            def proj_ln(wo_d, ATTN, bo_sb, resid, ln_g, ln_b, y_tag,
                        out_bf=None, out_f32=None, dma_out=None):
                """wo projection + residual + layernorm (feature-major)."""
                y = big.tile([128, NDT, SQ], f32r, tag=y_tag)
                pst1 = p_one.tile([128, SQ], f32, tag="pda")
                pst2 = p_one.tile([128, SQ], f32, tag="pdb")
                for dt in range(NDT):
                    wot = wcol.tile([128, NDT, 128], bf16, tag="wcol")
                    nc.sync.dma_start(out=wot[:], in_=wo_d[dt])
                    po = p_acc.tile([128, SQ], f32, tag="acc")
                    for ht in range(NDT):
                        nc.tensor.matmul(po[:], wot[:, ht, :], ATTN[:, ht, :],
                                         start=(ht == 0), stop=(ht == NDT - 1))
                    nc.vector.scalar_tensor_tensor(
                        out=y[:, dt, :], in0=po[:], scalar=bo_sb[:, dt:dt + 1],
                        in1=resid[:, dt, :], op0=OP.add, op1=OP.add)
                    sq = tmpp.tile([128, SQ], f32r, tag="sq")
                    nc.vector.tensor_mul(sq[:], y[:, dt, :], y[:, dt, :])
                    nc.tensor.matmul(pst1[0:1, :], ones_r[:, 0:1], y[:, dt, :],
                                     start=(dt == 0), stop=(dt == NDT - 1))
                    nc.tensor.matmul(pst2[0:1, :], ones_r[:, 0:1], sq[:],
                                     start=(dt == 0), stop=(dt == NDT - 1))
                ln_tail(pst1, pst2, y, ln_g, ln_b, out_bf, out_f32, dma_out)

            # ================= self-attention =================
            QT = big.tile([128, NHP, SQ], bf16, tag="qt")
            KT = big.tile([128, NHP, S], bf16, tag="kt")
            V2 = big.tile([128, NKC, 2, 512], bf16, tag="v2")
            ATTN = big.tile([128, NDT, SQ], bf16, tag="attn")

            def sa_steps(hp):
                st = q_steps(hp, w_sa_q, XQ, bq1_sb, QT) \
                    + k_steps(hp, w_sa_k, XT, KT)
                if hp % 4 == 0:
                    st += v_steps(hp // 4, w_sa_v, XT, V2)
                return st

            steps0 = sa_steps(0)
            steps0[0]()                        # Q(0) needs only XQ + wq
            nc.sync.dma_start(out=XT[:], in_=xt_full[:])
            for step in steps0[1:]:
                step()
            for hp in range(1, NHP):
                attention(hp - 1, QT, KT, V2, ATTN, True, sa_steps(hp))
            attention(NHP - 1, QT, KT, V2, ATTN, True)

            XR = big.tile([128, NDT, SQ], f32, tag="resid")
            nc.sync.dma_start(out=XR[:], in_=xr[:])
            OUT1B = big.tile([128, NDT, SQ], bf16, tag="outb")
            OUT1F = big.tile([128, NDT, SQ], f32, tag="resid")
            proj_ln(w_sa_o, ATTN, bo1_sb, XR, ln_sb[1][0], ln_sb[1][1],
                    y_tag="y", out_bf=OUT1B, out_f32=OUT1F)

            # ================= cross-attention =================
            # ENC reuses XT's slot (XT dead once SA K/V are built).
            ENC = big.tile([128, NDT, S], bf16, tag="xt")
            nc.sync.dma_start(out=ENC[:], in_=enc_t[:])
            KT2 = big.tile([128, NHP, S], bf16, tag="kt")
            V2c = big.tile([128, NKC, 2, 512], bf16, tag="v2")
            QT2 = big.tile([128, NHP, SQ], bf16, tag="qt")
            ATTN2 = big.tile([128, NDT, SQ], bf16, tag="attn")

            def ca_steps(hp):
                st = k_steps(hp, w_ca_k, ENC, KT2)
                if hp % 4 == 0:
                    st += v_steps(hp // 4, w_ca_v, ENC, V2c)
                st += q_steps(hp, w_ca_q, OUT1B, bq2_sb, QT2)
                return st

            for step in ca_steps(0):
                step()
            for hp in range(1, NHP):
                attention(hp - 1, QT2, KT2, V2c, ATTN2, False, ca_steps(hp))
            attention(NHP - 1, QT2, KT2, V2c, ATTN2, False)

            OUT2B = big.tile([128, NDT, SQ], bf16, tag="outb")
            OUT2F = big.tile([128, NDT, SQ], f32, tag="resid")
            proj_ln(w_ca_o, ATTN2, bo2_sb, OUT1F, ln_sb[2][0], ln_sb[2][1],
                    y_tag="y", out_bf=OUT2B, out_f32=OUT2F)

            # ================= feed-forward =================
            H1 = big.tile([128, NFT, SQ], bf16, tag="xt")  # reuse XT slot
            for ft in range(NFT):
                w1t = wcol.tile([128, NDT, 128], bf16, tag="wcol")
                nc.sync.dma_start(out=w1t[:], in_=w_ff1[ft])
                ph = p_acc.tile([128, SQ], f32, tag="acc")
                for dt in range(NDT):
                    nc.tensor.matmul(ph[:], w1t[:, dt, :], OUT2B[:, dt, :],
                                     start=(dt == 0), stop=(dt == NDT - 1))
                nc.scalar.activation(out=H1[:, ft, :], in_=ph[:], func=AF.Relu,
                                     bias=b1_sb[:, ft:ft + 1], scale=1.0)

            y3 = big.tile([128, NDT, SQ], f32r, tag="y")
            pst1 = p_one.tile([128, SQ], f32, tag="pda")
            pst2 = p_one.tile([128, SQ], f32, tag="pdb")
            for dt in range(NDT):
                w2t = wbig.tile([128, NFT, 128], bf16, tag="wbig")
                nc.sync.dma_start(out=w2t[:], in_=w_ff2[dt])
                pf = p_acc.tile([128, SQ], f32, tag="acc")
                for ft in range(NFT):
                    nc.tensor.matmul(pf[:], w2t[:, ft, :], H1[:, ft, :],
                                     start=(ft == 0), stop=(ft == NFT - 1))
                nc.vector.scalar_tensor_tensor(
                    out=y3[:, dt, :], in0=pf[:], scalar=b2_sb[:, dt:dt + 1],
                    in1=OUT2F[:, dt, :], op0=OP.add, op1=OP.add)
                sq = tmpp.tile([128, SQ], f32r, tag="sq")
                nc.vector.tensor_mul(sq[:], y3[:, dt, :], y3[:, dt, :])
                nc.tensor.matmul(pst1[0:1, :], ones_r[:, 0:1], y3[:, dt, :],
                                 start=(dt == 0), stop=(dt == NDT - 1))
                nc.tensor.matmul(pst2[0:1, :], ones_r[:, 0:1], sq[:],
                                 start=(dt == 0), stop=(dt == NDT - 1))
            ln_tail(pst1, pst2, y3, ln_sb[3][0], ln_sb[3][1], None, None, out_t)

    nc.compile()
    return nc


def _qrows(h):
    return np.concatenate(
        [np.arange(64 * (2 * t + h), 64 * (2 * t + h) + 64) for t in range(8)])


def _prepare_in_maps(inputs):
    f = np.float32
    di = np.asarray(inputs["decoder_input"], f)
    eo = np.asarray(inputs["encoder_output"], f)
    mask = np.asarray(inputs["mask"])

    def b16(a):
        return np.ascontiguousarray(a).astype(BF16)

    def wmat(w):  # (H, D, DH) -> (D, H*DH)
        return np.transpose(np.asarray(w, f), (1, 0, 2)).reshape(D, H * DH)

    def colmajor(w, no, co):  # [D_in, N_out] -> [no, 128, D_in/128, co]
        return w.reshape(w.shape[0] // 128, 128, no, co).transpose(2, 1, 0, 3)

    def pmajor(xt, n):  # [D, n] (feature-major) -> [128, NDT, n]
        return np.ascontiguousarray(
            xt.reshape(NDT, 128, n).transpose(1, 0, 2))

    shared = {}
    vecs = {}
    for p in ("sa", "ca"):
        shared[f"w_{p}_q"] = b16(colmajor(wmat(inputs[f"{p}_wq"]), NHP, 128))
        shared[f"w_{p}_k"] = b16(colmajor(wmat(inputs[f"{p}_wk"]), NHP, 128))
        shared[f"w_{p}_v"] = b16(colmajor(wmat(inputs[f"{p}_wv"]), 2, 512))
        wo = np.asarray(inputs[f"{p}_wo"], f)
        shared[f"w_{p}_o"] = b16(colmajor(wo, NDT, 128))
        vecs[f"bq_{p}"] = np.asarray(inputs[f"{p}_bq"], f).reshape(H * DH)
        bv = np.asarray(inputs[f"{p}_bv"], f).reshape(H * DH)
        vecs[f"bo_{p}"] = np.asarray(inputs[f"{p}_bo"], f) + bv @ wo
    shared["w_ff1"] = b16(colmajor(np.asarray(inputs["ff_w1"], f), NFT, 128))
    shared["w_ff2"] = b16(colmajor(np.asarray(inputs["ff_w2"], f), NDT, 128))

    def cols(v, n):  # [n*128] -> [128, n]
        return np.asarray(v, f).reshape(n, 128).T

    va = np.concatenate([
        cols(vecs["bq_sa"], NHP), cols(vecs["bq_ca"], NHP),
        cols(vecs["bo_sa"], NDT), cols(vecs["bo_ca"], NDT),
        cols(inputs["ff_b2"], NDT),
        cols(inputs["ln1_g"], NDT), cols(inputs["ln1_b"], NDT),
        cols(inputs["ln2_g"], NDT), cols(inputs["ln2_b"], NDT),
        cols(inputs["ln3_g"], NDT), cols(inputs["ln3_b"], NDT),
        cols(inputs["ff_b1"], NFT),
    ], axis=1)
    shared["v_all"] = np.ascontiguousarray(va, dtype=f)

    qr = {h: _qrows(h) for h in (0, 1)}
    in_maps = []
    for c in range(NCORES):
        b, h = divmod(c, 2)
        X = di[b]
        m = dict(shared)
        m["xt_full"] = b16(pmajor(X.T, S))
        Xq = X[qr[h]]
        m["xq"] = b16(pmajor(Xq.T, SQ))
        m["xr"] = np.ascontiguousarray(pmajor(Xq.T, SQ), dtype=f)
        m["enc_t"] = b16(pmajor(eo[b].T, S))
        mb = mask[b][qr[h]].astype(f)          # [SQ q, S k]
        slabs = np.zeros((NKC, 128, 64), f)
        for j in range(NKC):
            slabs[j] = mb[64 * j:64 * j + 64, 128 * j:128 * (j + 1)].T
        m["sa_mask"] = np.ascontiguousarray(
            slabs.transpose(1, 0, 2)).astype(BF16)
        in_maps.append(m)
    return in_maps


def _collect_output(results):
    qr = {h: _qrows(h) for h in (0, 1)}
    out = np.zeros((B, S, D), np.float32)
    for c in range(NCORES):
        b, h = divmod(c, 2)
        ot = np.asarray(results[c]["out_t"], np.float32).reshape(D, SQ)
        out[b, qr[h]] = ot.T
    return out


def kernel(**inputs):
    global _PROG
    if _PROG is None:
        _PROG = _build_program()
    from concourse.bass_utils import run_bass_kernel_spmd

    in_maps = _prepare_in_maps(inputs)
    res = run_bass_kernel_spmd(_PROG, in_maps, list(range(NCORES)))
    if res.exec_time_ns is not None:
        print(f"HW exec time: {res.exec_time_ns} ns")
    return _collect_output(res.results)


# revision 41
# speedup vs baseline: 1.0369x; 1.0369x over previous
"""Trainium2 Bass kernel for a transformer decoder block.

Shapes (hardcoded): B=4, S=1024, D=1024, H=16 heads, DH=64, FFN F=4096.

Sharding: 8 cores = 4 batches x 2 sequence-halves.  Core (b, h) handles
query rows {64*(2t+h)+r : t in 0..7, r in 0..63} of batch b (interleaved
64-row blocks so the causal-attention work per core is identical -> one
uniform SPMD program).  Each core recomputes the (small) K/V projections
it needs, so no collectives are required.

On-chip layout is feature-major ("transposed"): activations live as
[feature, token] so every matmul contraction sits on the partition axis.
The host pre-transposes inputs/weights and re-transposes the output.

Scheduling: engines execute their instruction streams in order, so each
attention head-pair's softmax (ScalarE-bound) is emitted with "filler"
projection matmul groups for the next head pair woven between its
k-chunks, keeping the PE busy while exps drain.
"""

import sys

if "/opt/trn_rl_repo" not in sys.path:
    sys.path.insert(0, "/opt/trn_rl_repo")

import numpy as np
import ml_dtypes

B, S, D, H, F, DH = 4, 1024, 1024, 16, 4096, 64
NCORES = 8
SQ = 512            # query rows per core
NDT = D // 128      # 8 d-tiles
NFT = F // 128      # 32 f-tiles
NHP = H // 2        # 8 head pairs
NKC = S // 128      # 8 k chunks
BF16 = ml_dtypes.bfloat16

_PROG = None


def _build_program():
    import concourse.mybir as mybir
    from concourse import bacc
    from concourse.tile import TileContext

    f32 = mybir.dt.float32
    bf16 = mybir.dt.bfloat16
    f32r = mybir.dt.float32r
    AF = mybir.ActivationFunctionType
    OP = mybir.AluOpType

    nc = bacc.Bacc("TRN2", target_bir_lowering=False, debug=False,
                   num_devices=NCORES)

    def din(name, shape, dt=bf16):
        return nc.dram_tensor(name, shape, dt, kind="ExternalInput")

    # activations, partition-major so each loads with ONE contiguous DMA
    xt_full = din("xt_full", [128, NDT, S])          # X^T (K/V source)
    xq = din("xq", [128, NDT, SQ])                   # X^T own q rows
    xr = din("xr", [128, NDT, SQ], f32)              # residual (f32)
    enc_t = din("enc_t", [128, NDT, S])              # encoder^T
    sa_mask = din("sa_mask", [128, NKC, 64])         # causal boundary slabs

    # weights staged host-side in exactly the sbuf tile layout
    w_sa_q = din("w_sa_q", [NHP, 128, NDT, 128])
    w_sa_k = din("w_sa_k", [NHP, 128, NDT, 128])
    w_sa_v = din("w_sa_v", [2, 128, NDT, 512])
    w_sa_o = din("w_sa_o", [NDT, 128, NDT, 128])
    w_ca_q = din("w_ca_q", [NHP, 128, NDT, 128])
    w_ca_k = din("w_ca_k", [NHP, 128, NDT, 128])
    w_ca_v = din("w_ca_v", [2, 128, NDT, 512])
    w_ca_o = din("w_ca_o", [NDT, 128, NDT, 128])
    w_ff1 = din("w_ff1", [NFT, 128, NDT, 128])
    w_ff2 = din("w_ff2", [NDT, 128, NFT, 128])

    # all small per-feature vectors concatenated: one DMA
    # cols: bq1 0:8 | bq2 8:16 | bo1 16:24 | bo2 24:32 | b2 32:40 |
    #       ln1g 40:48 | ln1b 48:56 | ln2g .. | ln3b 72:88 | b1 88:120
    NV = 120
    v_all = din("v_all", [128, NV], f32)

    out_t = nc.dram_tensor("out_t", [NDT, 128, SQ], f32, kind="ExternalOutput")

    with TileContext(nc) as tc:
        with tc.tile_pool(name="p_acc", bufs=2, space="PSUM") as p_acc, \
             tc.tile_pool(name="p_s", bufs=2, space="PSUM") as p_s, \
             tc.tile_pool(name="p_pav", bufs=2, space="PSUM") as p_pav, \
             tc.tile_pool(name="p_one", bufs=1, space="PSUM") as p_one, \
             tc.tile_pool(name="const", bufs=1) as cpool, \
             tc.tile_pool(name="big", bufs=1) as big, \
             tc.tile_pool(name="wcol", bufs=4) as wcol, \
             tc.tile_pool(name="wbig", bufs=2) as wbig, \
             tc.tile_pool(name="pt", bufs=4) as ptp, \
             tc.tile_pool(name="bc", bufs=2) as bcp, \
             tc.tile_pool(name="sm", bufs=1) as smp, \
             tc.tile_pool(name="tmp", bufs=2) as tmpp, \
             tc.tile_pool(name="outp", bufs=2) as outp:

            # ---------------- constants / small vectors ----------------
            ones_f = cpool.tile([128, 1], f32)
            nc.vector.memset(ones_f[:], 1.0)
            ones16 = cpool.tile([128, 1], bf16)
            nc.vector.tensor_copy(ones16[:], ones_f[:])
            # LN stat matmuls use 1/D so psum rows are mean / E[x^2] directly
            oned_f = cpool.tile([128, 1], f32)
            nc.vector.memset(oned_f[:], 1.0 / D)
            ones_r = cpool.tile([128, 1], f32r)
            nc.vector.tensor_copy(ones_r[:], oned_f[:])
            eps_t = cpool.tile([1, 1], f32)
            nc.vector.memset(eps_t[:], 1e-12)

            VA = cpool.tile([128, NV], f32)
            nc.sync.dma_start(out=VA[:], in_=v_all[:])
            bq1_sb, bq2_sb = VA[:, 0:8], VA[:, 8:16]
            bo1_sb, bo2_sb = VA[:, 16:24], VA[:, 24:32]
            b2_sb = VA[:, 32:40]
            ln_sb = {j: (VA[:, 40 + 16 * (j - 1):48 + 16 * (j - 1)],
                         VA[:, 48 + 16 * (j - 1):56 + 16 * (j - 1)])
                     for j in (1, 2, 3)}
            b1_sb = VA[:, 88:120]

            MS = cpool.tile([128, NKC, 64], bf16)
            nc.sync.dma_start(out=MS[:], in_=sa_mask[:])

            # ---------------- activation loads (XQ first: QT needs it) ----
            XQ = big.tile([128, NDT, SQ], bf16, tag="outb")
            nc.sync.dma_start(out=XQ[:], in_=xq[:])
            XT = big.tile([128, NDT, S], bf16, tag="xt")

            # ---------------- filler-step builders ----------------
            # Each returned closure emits one psum matmul group; they are
            # woven between attention k-chunks to keep the PE fed while the
            # ScalarE runs the softmax exps.
            def q_steps(hp, wq_d, src_q, bq_sb, QT):
                def run():
                    wqt = wcol.tile([128, NDT, 128], bf16, tag="wcol")
                    nc.sync.dma_start(out=wqt[:], in_=wq_d[hp])
                    pq = p_acc.tile([128, SQ], f32, tag="acc")
                    for dt in range(NDT):
                        nc.tensor.matmul(pq[:], wqt[:, dt, :], src_q[:, dt, :],
                                         start=(dt == 0), stop=(dt == NDT - 1))
                    nc.vector.tensor_scalar_add(QT[:, hp, :], pq[:],
                                                bq_sb[:, hp:hp + 1])
                return [run]

            def k_steps(hp, wk_d, src_kv, KT):
                cell = {}

                def run_kh(kh):
                    def run():
                        if kh == 0:
                            cell["w"] = wcol.tile([128, NDT, 128], bf16,
                                                  tag="wcol", name="wkt")
                            nc.sync.dma_start(out=cell["w"][:], in_=wk_d[hp])
                        wkt = cell["w"]
                        pk = p_acc.tile([128, 512], f32, tag="acc")
                        for dt in range(NDT):
                            nc.tensor.matmul(
                                pk[:], wkt[:, dt, :],
                                src_kv[:, dt, 512 * kh:512 * (kh + 1)],
                                start=(dt == 0), stop=(dt == NDT - 1))
                        nc.vector.tensor_copy(
                            KT[:, hp, 512 * kh:512 * (kh + 1)], pk[:])
                    return run
                return [run_kh(0), run_kh(1)]

            def v_steps(g, wv_d, src_kv, V2):
                cell = {}

                def run_kc(kc):
                    def run():
                        if kc == 0:
                            cell["w"] = wbig.tile([128, NDT, 512], bf16,
                                                  tag="wbig", name="wvt")
                            nc.sync.dma_start(out=cell["w"][:], in_=wv_d[g])
                        wvt = cell["w"]
                        pv = p_acc.tile([128, 512], f32, tag="acc")
                        for dt in range(NDT):
                            nc.tensor.matmul(
                                pv[:], src_kv[:, dt, 128 * kc:128 * (kc + 1)],
                                wvt[:, dt, :],
                                start=(dt == 0), stop=(dt == NDT - 1))
                        nc.vector.tensor_copy(V2[:, kc, g, :], pv[:])
                    return run
                return [run_kc(kc) for kc in range(NKC)]

            def attention(hp, QT, KT, V2, ATTN, causal, fillers=()):
                pav = p_pav.tile([128, SQ], f32, tag="pav")
                pda = p_one.tile([128, SQ], f32, tag="pda")
                pdb = p_one.tile([128, SQ], f32, tag="pdb")
                g, m = hp // 4, hp % 4
                fillers = list(fillers)
                fi = 0
                for j in range(NKC):
                    n0 = 64 * j if causal else 0
                    sa_ = p_s.tile([128, SQ], f32, tag="s")
                    sb_ = p_s.tile([128, SQ], f32, tag="s")
                    ks = slice(128 * j, 128 * (j + 1))
                    nc.tensor.matmul(sa_[:, n0:SQ], KT[0:64, hp, ks],
                                     QT[0:64, hp, n0:SQ], start=True, stop=True)
                    nc.tensor.matmul(sb_[:, n0:SQ], KT[64:128, hp, ks],
                                     QT[64:128, hp, n0:SQ], start=True,
                                     stop=True)
                    pta = ptp.tile([128, SQ], bf16, tag="pt")
                    ptb = ptp.tile([128, SQ], bf16, tag="pt")
                    nc.scalar.activation(out=pta[:, n0:SQ], in_=sa_[:, n0:SQ],
                                         func=AF.Exp, scale=0.125)
                    nc.scalar.activation(out=ptb[:, n0:SQ], in_=sb_[:, n0:SQ],
                                         func=AF.Exp, scale=0.125)
                    if causal:
                        nc.vector.tensor_mul(pta[:, n0:n0 + 64],
                                             pta[:, n0:n0 + 64], MS[:, j, :])
                        nc.vector.tensor_mul(ptb[:, n0:n0 + 64],
                                             ptb[:, n0:n0 + 64], MS[:, j, :])
                    # fillers go HERE (between scores and AV) so the PE chews
                    # on them while ScalarE exps this chunk
                    while fi < len(fillers) and fi * NKC < (j + 1) * len(fillers):
                        fillers[fi]()
                        fi += 1
                    st, sp = (j == 0), (j == NKC - 1)
                    nc.tensor.matmul(pav[0:64, n0:SQ],
                                     V2[:, j, g, 128 * m:128 * m + 64],
                                     pta[:, n0:SQ], start=st, stop=sp)
                    nc.tensor.matmul(pav[64:128, n0:SQ],
                                     V2[:, j, g, 128 * m + 64:128 * (m + 1)],
                                     ptb[:, n0:SQ], start=st, stop=sp)
                    nc.tensor.matmul(pda[0:1, n0:SQ], ones16[:, 0:1],
                                     pta[:, n0:SQ], start=st, stop=sp)
                    nc.tensor.matmul(pdb[0:1, n0:SQ], ones16[:, 0:1],
                                     ptb[:, n0:SQ], start=st, stop=sp)
                while fi < len(fillers):
                    fillers[fi]()
                    fi += 1
                ra = smp.tile([1, SQ], f32, tag="ra")
                rb = smp.tile([1, SQ], f32, tag="rb")
                nc.vector.reciprocal_approx_fast(out=ra[:], in_=pda[0:1, :])
                nc.vector.reciprocal_approx_fast(out=rb[:], in_=pdb[0:1, :])
                RA = bcp.tile([128, SQ], f32, tag="bc")
                RB = bcp.tile([128, SQ], f32, tag="bc")
                nc.gpsimd.partition_broadcast(RA[:], ra[:])
                nc.gpsimd.partition_broadcast(RB[:], rb[:])
                nc.vector.tensor_mul(ATTN[0:64, hp, :], pav[0:64, :],
                                     RA[0:64, :])
                nc.vector.tensor_mul(ATTN[64:128, hp, :], pav[64:128, :],
                                     RB[64:128, :])

            def ln_tail(pst1, pst2, y, ln_g, ln_b, out_bf, out_f32, dma_out):
                m1 = smp.tile([1, SQ], f32, tag="m1")
                nc.vector.tensor_copy(m1[:], pst1[0:1, :])  # mean (ones=1/D)
                MB = bcp.tile([128, SQ], f32, tag="bc")
                nc.gpsimd.partition_broadcast(MB[:], m1[:])
                sq1 = smp.tile([1, SQ], f32, tag="sq1")
                nc.vector.tensor_mul(sq1[:], m1[:], m1[:])
                varp = smp.tile([1, SQ], f32, tag="varp")
                nc.vector.tensor_sub(varp[:], pst2[0:1, :], sq1[:])
                sv = smp.tile([1, SQ], f32, tag="sv")
                nc.scalar.activation(out=sv[:], in_=varp[:], func=AF.Sqrt,
                                     bias=eps_t[:], scale=float(D) / (D - 1))
                rstd = smp.tile([1, SQ], f32, tag="rstd")
                nc.vector.reciprocal_approx_fast(out=rstd[:], in_=sv[:])
                RS = bcp.tile([128, SQ], f32, tag="bc")
                nc.gpsimd.partition_broadcast(RS[:], rstd[:])
                for dt in range(NDT):
                    t1 = tmpp.tile([128, SQ], f32, tag="lnt")
                    nc.vector.tensor_sub(t1[:], y[:, dt, :], MB[:])
                    nc.vector.tensor_mul(t1[:], t1[:], RS[:])
                    g_ap = ln_g[:, dt:dt + 1]
                    b_ap = ln_b[:, dt:dt + 1]
                    use_act = (dt % 2 == 1)  # split affine between ACT / DVE
                    if dma_out is not None:
                        od = outp.tile([128, SQ], f32, tag="od")
                        if use_act:
                            nc.scalar.activation(out=od[:], in_=t1[:],
                                                 func=AF.Identity,
                                                 bias=b_ap, scale=g_ap)
                        else:
                            nc.vector.tensor_scalar(od[:], t1[:], g_ap, b_ap,
                                                    OP.mult, OP.add)
                        nc.sync.dma_start(out=dma_out[dt], in_=od[:])
                    else:
                        if use_act:
                            nc.scalar.activation(out=out_f32[:, dt, :],
                                                 in_=t1[:], func=AF.Identity,
                                                 bias=b_ap, scale=g_ap)
                            nc.scalar.activation(out=out_bf[:, dt, :],
                                                 in_=t1[:], func=AF.Identity,
                                                 bias=b_ap, scale=g_ap)
                        else:
                            nc.vector.tensor_scalar(out_f32[:, dt, :], t1[:],
                                                    g_ap, b_ap,
                                                    OP.mult, OP.add)
                            nc.vector.tensor_copy(out_bf[:, dt, :],
                                                  out_f32[:, dt, :])

            def proj_ln(wo_d, ATTN, bo_sb, resid, ln_g, ln_b, y_tag,
                        out_bf=None, out_f32=None, dma_out=None):
                """wo projection + residual + layernorm (feature-major)."""
                y = big.tile([128, NDT, SQ], f32r, tag=y_tag)
                pst1 = p_one.tile([128, SQ], f32, tag="pda")
                pst2 = p_one.tile([128, SQ], f32, tag="pdb")
                for dt in range(NDT):
                    wot = wcol.tile([128, NDT, 128], bf16, tag="wcol")
                    nc.sync.dma_start(out=wot[:], in_=wo_d[dt])
                    po = p_acc.tile([128, SQ], f32, tag="acc")
                    for ht in range(NDT):
                        nc.tensor.matmul(po[:], wot[:, ht, :], ATTN[:, ht, :],
                                         start=(ht == 0), stop=(ht == NDT - 1))
                    nc.vector.scalar_tensor_tensor(
                        out=y[:, dt, :], in0=po[:], scalar=bo_sb[:, dt:dt + 1],
                        in1=resid[:, dt, :], op0=OP.add, op1=OP.add)
                    sq = tmpp.tile([128, SQ], f32r, tag="sq")
                    nc.vector.tensor_mul(sq[:], y[:, dt, :], y[:, dt, :])
                    nc.tensor.matmul(pst1[0:1, :], ones_r[:, 0:1], y[:, dt, :],
                                     start=(dt == 0), stop=(dt == NDT - 1))
                    nc.tensor.matmul(pst2[0:1, :], ones_r[:, 0:1], sq[:],
                                     start=(dt == 0), stop=(dt == NDT - 1))
                ln_tail(pst1, pst2, y, ln_g, ln_b, out_bf, out_f32, dma_out)

            # ================= self-attention =================
            QT = big.tile([128, NHP, SQ], bf16, tag="qt")
            KT = big.tile([128, NHP, S], bf16, tag="kt")
            V2 = big.tile([128, NKC, 2, 512], bf16, tag="v2")
            ATTN = big.tile([128, NDT, SQ], bf16, tag="attn")

            def sa_steps(hp):
                st = q_steps(hp, w_sa_q, XQ, bq1_sb, QT) \
                    + k_steps(hp, w_sa_k, XT, KT)
                if hp % 4 == 0:
                    st += v_steps(hp // 4, w_sa_v, XT, V2)
                return st

            steps0 = sa_steps(0)
            steps0[0]()                        # Q(0) needs only XQ + wq
            nc.sync.dma_start(out=XT[:], in_=xt_full[:])
            for step in steps0[1:]:
                step()
            for hp in range(1, NHP):
                attention(hp - 1, QT, KT, V2, ATTN, True, sa_steps(hp))
            attention(NHP - 1, QT, KT, V2, ATTN, True)

            XR = big.tile([128, NDT, SQ], f32, tag="resid")
            nc.sync.dma_start(out=XR[:], in_=xr[:])
            OUT1B = big.tile([128, NDT, SQ], bf16, tag="outb")
            OUT1F = big.tile([128, NDT, SQ], f32, tag="resid")
            proj_ln(w_sa_o, ATTN, bo1_sb, XR, ln_sb[1][0], ln_sb[1][1],
                    y_tag="y", out_bf=OUT1B, out_f32=OUT1F)

            # ================= cross-attention =================
            # ENC reuses XT's slot (XT dead once SA K/V are built).
            ENC = big.tile([128, NDT, S], bf16, tag="xt")
            nc.sync.dma_start(out=ENC[:], in_=enc_t[:])
            KT2 = big.tile([128, NHP, S], bf16, tag="kt")
            V2c = big.tile([128, NKC, 2, 512], bf16, tag="v2")
            QT2 = big.tile([128, NHP, SQ], bf16, tag="qt")
            ATTN2 = big.tile([128, NDT, SQ], bf16, tag="attn")

            def ca_steps(hp):
                st = k_steps(hp, w_ca_k, ENC, KT2)
                if hp % 4 == 0:
                    st += v_steps(hp // 4, w_ca_v, ENC, V2c)
                st += q_steps(hp, w_ca_q, OUT1B, bq2_sb, QT2)
                return st

            for step in ca_steps(0):
                step()
            for hp in range(1, NHP):
                attention(hp - 1, QT2, KT2, V2c, ATTN2, False, ca_steps(hp))
            attention(NHP - 1, QT2, KT2, V2c, ATTN2, False)

            OUT2B = big.tile([128, NDT, SQ], bf16, tag="outb")
            OUT2F = big.tile([128, NDT, SQ], f32, tag="resid")
            proj_ln(w_ca_o, ATTN2, bo2_sb, OUT1F, ln_sb[2][0], ln_sb[2][1],
                    y_tag="y", out_bf=OUT2B, out_f32=OUT2F)

            # ================= feed-forward =================
            H1 = big.tile([128, NFT, SQ], bf16, tag="xt")  # reuse XT slot
            for ft in range(NFT):
                w1t = wcol.tile([128, NDT, 128], bf16, tag="wcol")
                nc.sync.dma_start(out=w1t[:], in_=w_ff1[ft])
                ph = p_acc.tile([128, SQ], f32, tag="acc")
                for dt in range(NDT):
                    nc.tensor.matmul(ph[:], w1t[:, dt, :], OUT2B[:, dt, :],
                                     start=(dt == 0), stop=(dt == NDT - 1))
                nc.scalar.activation(out=H1[:, ft, :], in_=ph[:], func=AF.Relu,
                                     bias=b1_sb[:, ft:ft + 1], scale=1.0)

            y3 = big.tile([128, NDT, SQ], f32r, tag="y")
            pst1 = p_one.tile([128, SQ], f32, tag="pda")
            pst2 = p_one.tile([128, SQ], f32, tag="pdb")
            for dt in range(NDT):
                w2t = wbig.tile([128, NFT, 128], bf16, tag="wbig")
                nc.sync.dma_start(out=w2t[:], in_=w_ff2[dt])
                pf = p_acc.tile([128, SQ], f32, tag="acc")
                for ft in range(NFT):
                    nc.tensor.matmul(pf[:], w2t[:, ft, :], H1[:, ft, :],
                                     start=(ft == 0), stop=(ft == NFT - 1))
                nc.vector.scalar_tensor_tensor(
                    out=y3[:, dt, :], in0=pf[:], scalar=b2_sb[:, dt:dt + 1],
                    in1=OUT2F[:, dt, :], op0=OP.add, op1=OP.add)
                sq = tmpp.tile([128, SQ], f32r, tag="sq")
                nc.vector.tensor_mul(sq[:], y3[:, dt, :], y3[:, dt, :])
                nc.tensor.matmul(pst1[0:1, :], ones_r[:, 0:1], y3[:, dt, :],
                                 start=(dt == 0), stop=(dt == NDT - 1))
                nc.tensor.matmul(pst2[0:1, :], ones_r[:, 0:1], sq[:],
                                 start=(dt == 0), stop=(dt == NDT - 1))
            ln_tail(pst1, pst2, y3, ln_sb[3][0], ln_sb[3][1], None, None, out_t)

    nc.compile()
    return nc


def _qrows(h):
    return np.concatenate(
        [np.arange(64 * (2 * t + h), 64 * (2 * t + h) + 64) for t in range(8)])


def _prepare_in_maps(inputs):
    f = np.float32
    di = np.asarray(inputs["decoder_input"], f)
    eo = np.asarray(inputs["encoder_output"], f)
    mask = np.asarray(inputs["mask"])

    def b16(a):
        return np.ascontiguousarray(a).astype(BF16)

    def wmat(w):  # (H, D, DH) -> (D, H*DH)
        return np.transpose(np.asarray(w, f), (1, 0, 2)).reshape(D, H * DH)

    def colmajor(w, no, co):  # [D_in, N_out] -> [no, 128, D_in/128, co]
        return w.reshape(w.shape[0] // 128, 128, no, co).transpose(2, 1, 0, 3)

    def pmajor(xt, n):  # [D, n] (feature-major) -> [128, NDT, n]
        return np.ascontiguousarray(
            xt.reshape(NDT, 128, n).transpose(1, 0, 2))

    shared = {}
    vecs = {}
    for p in ("sa", "ca"):
        shared[f"w_{p}_q"] = b16(colmajor(wmat(inputs[f"{p}_wq"]), NHP, 128))
        shared[f"w_{p}_k"] = b16(colmajor(wmat(inputs[f"{p}_wk"]), NHP, 128))
        shared[f"w_{p}_v"] = b16(colmajor(wmat(inputs[f"{p}_wv"]), 2, 512))
        wo = np.asarray(inputs[f"{p}_wo"], f)
        shared[f"w_{p}_o"] = b16(colmajor(wo, NDT, 128))
        vecs[f"bq_{p}"] = np.asarray(inputs[f"{p}_bq"], f).reshape(H * DH)
        bv = np.asarray(inputs[f"{p}_bv"], f).reshape(H * DH)
        vecs[f"bo_{p}"] = np.asarray(inputs[f"{p}_bo"], f) + bv @ wo
    shared["w_ff1"] = b16(colmajor(np.asarray(inputs["ff_w1"], f), NFT, 128))
    shared["w_ff2"] = b16(colmajor(np.asarray(inputs["ff_w2"], f), NDT, 128))

    def cols(v, n):  # [n*128] -> [128, n]
        return np.asarray(v, f).reshape(n, 128).T

    va = np.concatenate([
        cols(vecs["bq_sa"], NHP), cols(vecs["bq_ca"], NHP),
        cols(vecs["bo_sa"], NDT), cols(vecs["bo_ca"], NDT),
        cols(inputs["ff_b2"], NDT),
        cols(inputs["ln1_g"], NDT), cols(inputs["ln1_b"], NDT),
        cols(inputs["ln2_g"], NDT), cols(inputs["ln2_b"], NDT),
        cols(inputs["ln3_g"], NDT), cols(inputs["ln3_b"], NDT),
        cols(inputs["ff_b1"], NFT),
    ], axis=1)
    shared["v_all"] = np.ascontiguousarray(va, dtype=f)

    qr = {h: _qrows(h) for h in (0, 1)}
    in_maps = []
    for c in range(NCORES):
        b, h = divmod(c, 2)
        X = di[b]
        m = dict(shared)
        m["xt_full"] = b16(pmajor(X.T, S))
        Xq = X[qr[h]]
        m["xq"] = b16(pmajor(Xq.T, SQ))
        m["xr"] = np.ascontiguousarray(pmajor(Xq.T, SQ), dtype=f)
        m["enc_t"] = b16(pmajor(eo[b].T, S))
        mb = mask[b][qr[h]].astype(f)          # [SQ q, S k]
        slabs = np.zeros((NKC, 128, 64), f)
        for j in range(NKC):
            slabs[j] = mb[64 * j:64 * j + 64, 128 * j:128 * (j + 1)].T
        m["sa_mask"] = np.ascontiguousarray(
            slabs.transpose(1, 0, 2)).astype(BF16)
        in_maps.append(m)
    return in_maps


def _collect_output(results):
    qr = {h: _qrows(h) for h in (0, 1)}
    out = np.zeros((B, S, D), np.float32)
    for c in range(NCORES):
        b, h = divmod(c, 2)
        ot = np.asarray(results[c]["out_t"], np.float32).reshape(D, SQ)
        out[b, qr[h]] = ot.T
    return out


def kernel(**inputs):
    global _PROG
    if _PROG is None:
        _PROG = _build_program()
    from concourse.bass_utils import run_bass_kernel_spmd

    in_maps = _prepare_in_maps(inputs)
    res = run_bass_kernel_spmd(_PROG, in_maps, list(range(NCORES)))
    if res.exec_time_ns is not None:
        print(f"HW exec time: {res.exec_time_ns} ns")
    return _collect_output(res.results)
